# revision 32
# baseline (speedup 1.0000x reference)
"""Trainium2 Bass kernel for AInnoFaceLoss (anchor-matching detection loss).

Sharding: anchors (C) are split across the 8 NeuronCores; every core
handles its 25088-anchor stripe against ALL 8 images' ground-truth
boxes.  This puts 8 images x 64 GT = 512 elements on the free axis of
every pairwise op (vs 64 for batch sharding), amortizing the ~58-cycle
DVE instruction overhead, while anchor coordinates stay per-partition
scalars (free operands of tensor_scalar / scalar_tensor_tensor).

Phase A per anchor-block t (128 anchors on partitions, 512 GT on free):
pairwise IoU, per-image max -> ts, one-hot(argmax) -> matched box tb.
The division inter/union runs as exp(ln - ln) on the Scalar engine's
LUT pipe (single combined ln+exp table; the allocator is patched so it
never reloads tables).  tb gathering runs on the otherwise-idle
TensorEngine: PE-transpose of the one-hot mask + tiny matmuls with the
GT coordinate lists.

Phase B (per image, anchors along free): sigmoid-focal terms via
ln/exp identities, threshold counts, masked -log(elementwise IoU).

Each core emits 6 partial sums; host combines (global counts, the
final four divisions) - order-free reductions, so the anchor
permutation from sharding never needs to be undone.
"""
from contextlib import ExitStack

import numpy as np

import concourse.bass as bass
import concourse.tile as tile
from concourse import bacc, mybir
from concourse.bass_utils import run_bass_kernel_spmd
from concourse.masks import make_identity

B, C, K = 8, 200000, 64
P = 128
NTC = 196  # anchor blocks per core
PC = P * NTC  # 25088 anchors per core
CPAD = 8 * PC  # 200704
G = B * K  # 512 gt across all images
FS_HI, SS_HI = 0.7, 0.5
DT = mybir.dt.float32
AL = mybir.AluOpType
AF = mybir.ActivationFunctionType

_CACHE = {}


def _patch_act_tables():
    """Keep ln/exp/abs only in the one table that holds all three;
    otherwise the allocator ping-pongs tables with a ~1.3us
    ACT_TABLE_LOAD between every Ln and Exp instruction."""
    from concourse import hw_specs

    orig = hw_specs.get_activation_tables

    def only_lnexp(arch):
        t = dict(orig(arch))
        key = "natural_log_exp_and_others"
        strip = {AF.Ln, AF.Exp, AF.Abs}
        for k in t:
            if k != key:
                t[k] = t[k] - strip
        return t

    bacc.get_activation_tables = only_lnexp


def _build_kernel():
    _patch_act_tables()
    nc = bacc.Bacc(
        "TRN2",
        target_bir_lowering=False,
        debug=False,
        enable_asserts=False,
        num_devices=8,
    )
    anc_d = nc.dram_tensor("anc", [PC, 4], DT, kind="ExternalInput").ap()
    gt_d = nc.dram_tensor("gt", [B, K, 4], DT, kind="ExternalInput").ap()
    fs_d = nc.dram_tensor("fs", [B, PC, 6], DT, kind="ExternalInput").ap()
    ss_d = nc.dram_tensor("ss", [B, PC, 6], DT, kind="ExternalInput").ap()
    out_d = nc.dram_tensor("out", [P, 8], DT, kind="ExternalOutput").ap()

    with tile.TileContext(nc) as tc:
        with ExitStack() as ctx:
            _body(ctx, tc, anc_d, gt_d, fs_d, ss_d, out_d)
    nc.compile()
    return nc


def _body(ctx, tc, anc_d, gt_d, fs_d, ss_d, out_d):
    nc = tc.nc

    persist = ctx.enter_context(tc.tile_pool(name="persist", bufs=1))
    psum = ctx.enter_context(tc.tile_pool(name="psum", bufs=1, space="PSUM"))
    psA = ctx.enter_context(tc.tile_pool(name="psA", bufs=2, space="PSUM"))
    atmp = ctx.enter_context(tc.tile_pool(name="atmp", bufs=3))
    btmp = ctx.enter_context(tc.tile_pool(name="btmp", bufs=1))
    bload = ctx.enter_context(tc.tile_pool(name="bload", bufs=1))

    bias0 = persist.tile([P, 1], DT, tag="bias0")
    nc.vector.memset(bias0[:], 0.0)
    biasEps = persist.tile([P, 1], DT, tag="biasEps")
    nc.vector.memset(biasEps[:], 1e-30)
    bias1 = persist.tile([P, 1], DT, tag="bias1")
    nc.vector.memset(bias1[:], 1.0)

    ident = persist.tile([P, P], DT, tag="ident")
    make_identity(nc, ident[:])

    # ---- anchors (anchor index within stripe = p*NTC + t) ----
    A = persist.tile([P, NTC, 4], DT, tag="A")
    nc.gpsimd.dma_start(A[:], anc_d.rearrange("(p t) c -> p t c", p=P))
    X2 = persist.tile([P, NTC], DT, tag="X2")
    Y2 = persist.tile([P, NTC], DT, tag="Y2")
    AR = persist.tile([P, NTC], DT, tag="AR")
    nc.vector.tensor_add(X2[:], A[:, :, 0], A[:, :, 2])
    nc.vector.tensor_add(Y2[:], A[:, :, 1], A[:, :, 3])
    nc.vector.tensor_mul(AR[:], A[:, :, 2], A[:, :, 3])

    # ---- gt broadcast across partitions: (128, 512) coordinate tiles ----
    grow = persist.tile([1, G * 4], DT, tag="grow")
    nc.gpsimd.dma_start(grow[:], gt_d.rearrange("b k c -> (b k c)")[None, :])
    ones = persist.tile([1, P], DT, tag="ones")
    nc.vector.memset(ones[:], 1.0)
    gps = psum.tile([P, G * 4], DT, tag="gps")
    for i in range(4):
        nc.tensor.matmul(
            gps[:, i * 512 : (i + 1) * 512], ones[:], grow[:, i * 512 : (i + 1) * 512]
        )
    gb = gps[:].rearrange("p (g c) -> p g c", g=G)
    GX = persist.tile([P, G], DT, tag="GX")
    GY = persist.tile([P, G], DT, tag="GY")
    GW = persist.tile([P, G], DT, tag="GW")
    GH = persist.tile([P, G], DT, tag="GH")
    nc.vector.tensor_copy(GX[:], gb[:, :, 0])
    nc.vector.tensor_copy(GY[:], gb[:, :, 1])
    nc.vector.tensor_copy(GW[:], gb[:, :, 2])
    nc.vector.tensor_copy(GH[:], gb[:, :, 3])
    BX2 = persist.tile([P, G], DT, tag="BX2")
    BY2 = persist.tile([P, G], DT, tag="BY2")
    SB = persist.tile([P, G], DT, tag="SB")
    nc.vector.tensor_add(BX2[:], GX[:], GW[:])
    nc.vector.tensor_add(BY2[:], GY[:], GH[:])
    nc.vector.tensor_mul(SB[:], GW[:], GH[:])

    # gt per-image coordinate lists for the tb matmuls, duplicated on
    # partitions 64-127 so either half can serve as matmul rhs: (128, B, 4)
    gtm = persist.tile([2 * K, B, 4], DT, tag="gtm")
    nc.gpsimd.dma_start(gtm[:K], gt_d.rearrange("b k c -> k b c"))
    nc.gpsimd.dma_start(gtm[K:], gt_d.rearrange("b k c -> k b c"))

    # ---- phase A outputs ----
    TS = persist.tile([P, NTC, B], DT, tag="TS")
    TB4 = persist.tile([P, NTC, B * 4], DT, tag="TB4")
    if not (DO_TB and DO_MM):
        nc.vector.memset(TB4[:], 1.0)

    # ---- phase A ----
    for t in range(NTC):
        ax1 = A[:, t, 0:1]
        ay1 = A[:, t, 1:2]
        ax2 = X2[:, t : t + 1]
        ay2 = Y2[:, t : t + 1]
        sa = AR[:, t : t + 1]

        t2 = atmp.tile([P, G], DT, tag="t2")
        nc.vector.tensor_scalar(t2[:], GX[:], ax1, None, AL.max)
        u1 = atmp.tile([P, G], DT, tag="u1")
        nc.vector.tensor_scalar(u1[:], BX2[:], ax2, None, AL.min)
        w = atmp.tile([P, G], DT, tag="w")
        nc.gpsimd.tensor_sub(w[:], u1[:], t2[:])
        t4 = atmp.tile([P, G], DT, tag="t4")
        nc.vector.tensor_scalar(t4[:], GY[:], ay1, None, AL.max)
        u2 = atmp.tile([P, G], DT, tag="u2")
        nc.vector.tensor_scalar(u2[:], BY2[:], ay2, None, AL.min)
        h = atmp.tile([P, G], DT, tag="h")
        nc.gpsimd.tensor_sub(h[:], u2[:], t4[:])
        hr = atmp.tile([P, G], DT, tag="hr")
        nc.vector.tensor_scalar(hr[:], h[:], 0.0, None, AL.max)
        wr = atmp.tile([P, G], DT, tag="wr")
        nc.vector.tensor_scalar(wr[:], w[:], 0.0, None, AL.max)
        inter = atmp.tile([P, G], DT, tag="inter")
        nc.gpsimd.tensor_mul(inter[:], wr[:], hr[:])
        un = atmp.tile([P, G], DT, tag="un")
        nc.vector.scalar_tensor_tensor(un[:], SB[:], sa, inter[:], AL.add, AL.subtract)

        lnI = atmp.tile([P, G], DT, tag="lnI")
        nc.scalar.activation(lnI[:], inter[:], AF.Ln, bias=biasEps[:])
        lnU = atmp.tile([P, G], DT, tag="lnU")
        nc.scalar.activation(lnU[:], un[:], AF.Ln, bias=bias0[:])
        d = atmp.tile([P, G], DT, tag="d")
        nc.gpsimd.tensor_sub(d[:], lnI[:], lnU[:])
        iou = atmp.tile([P, G], DT, tag="iou")
        nc.scalar.activation(iou[:], d[:], AF.Exp, bias=bias0[:])

        iou3 = iou[:].rearrange("p (b k) -> p b k", b=B)
        nc.vector.tensor_reduce(TS[:, t, :], iou3, axis=mybir.AxisListType.X, op=AL.max)
        m = atmp.tile([P, G], DT, tag="m")
        tsb = TS[:, t : t + 1, :].rearrange("p o b -> p b o").to_broadcast([P, B, K])
        nc.vector.tensor_tensor(
            m[:].rearrange("p (b k) -> p b k", b=B), iou3, tsb, AL.is_ge
        )

        # tb = onehot @ gt, on the TensorEngine
        mts = atmp.tile([P, G], DT, tag="mts")
        for g in range(4):
            mtp = psA.tile([P, P], DT, tag="mtp", name="mtp")
            nc.tensor.transpose(mtp[:], m[:, g * P : (g + 1) * P], ident[:])
            nc.scalar.copy(mts[:, g * P : (g + 1) * P], mtp[:])
        if DO_MM:
            for img in range(B):
                g, half = img // 2, img % 2
                tbp = psA.tile([P, 4], DT, tag="tbp", name="tbp")
                nc.tensor.matmul(
                    tbp[:],
                    mts[half * K : (half + 1) * K, g * P : (g + 1) * P],
                    gtm[half * K : (half + 1) * K, img, :],
                )
                nc.scalar.copy(TB4[:, t, img * 4 : (img + 1) * 4], tbp[:])
        else:
            nc.vector.tensor_copy(TB4[:, t, 0:16], mts[:, 0:16])

    # ---- phase B ----
    ACC = persist.tile([P, 8], DT, tag="ACC")
    FOC = [persist.tile([P, B], DT, tag=f"FOC{i}", name=f"FOC{i}") for i in range(2)]
    CNT = [persist.tile([P, B], DT, tag=f"CNT{i}", name=f"CNT{i}") for i in range(2)]
    IOL = [persist.tile([P, B], DT, tag=f"IOL{i}", name=f"IOL{i}") for i in range(2)]

    F = NTC
    for img in range(B):
        ts_c = TS[:, :, img]
        tbx = TB4[:, :, img * 4 + 0]
        tby = TB4[:, :, img * 4 + 1]
        tbw = TB4[:, :, img * 4 + 2]
        tbh = TB4[:, :, img * 4 + 3]

        def tmp(tag):
            return btmp.tile([P, F], DT, tag=tag, name=tag)

        tx2 = tmp("tx2")
        nc.vector.tensor_add(tx2[:], tbx, tbw)
        ty2 = tmp("ty2")
        nc.vector.tensor_add(ty2[:], tby, tbh)
        ta = tmp("ta")
        nc.vector.tensor_mul(ta[:], tbw, tbh)

        for si, (src_d, tau) in enumerate(((fs_d, FS_HI), (ss_d, SS_HI))):
            pr = bload.tile([P, F, 6], DT, tag=f"prop{si}", name=f"prop{si}")
            nc.gpsimd.dma_start(pr[:], src_d[img].rearrange("(p t) c -> p t c", p=P))
            px = pr[:, :, 0]
            py = pr[:, :, 1]
            pw = pr[:, :, 2]
            ph = pr[:, :, 3]
            lg = pr[:, :, 4]

            # focal loss via ln/exp only:
            #   lp = ln(1+exp(-|l|)); softplus(l) = relu(l)+lp
            #   p = sigmoid(l) = exp(min(l,0) - lp)
            al = tmp("al")
            nc.scalar.activation(al[:], lg, AF.Abs, bias=bias0[:])
            ex = tmp("ex")
            nc.scalar.activation(ex[:], al[:], AF.Exp, bias=bias0[:], scale=-1.0)
            lp = tmp("lpb")
            nc.scalar.activation(lp[:], ex[:], AF.Ln, bias=bias1[:])
            parg = tmp("pargb")
            nc.vector.scalar_tensor_tensor(parg[:], lg, 0.0, lp[:], AL.min, AL.subtract)
            pp = tmp("ppb")
            nc.scalar.activation(pp[:], parg[:], AF.Exp, bias=bias0[:])
            sp = tmp("spb")
            nc.vector.scalar_tensor_tensor(sp[:], lg, 0.0, lp[:], AL.max, AL.add)
            lt = tmp("ltb")
            nc.vector.tensor_mul(lt[:], lg, ts_c)
            ce = tmp("ceb")
            nc.vector.tensor_sub(ce[:], sp[:], lt[:])
            pt = tmp("ptb")
            nc.vector.tensor_mul(pt[:], pp[:], ts_c)
            s1 = tmp("s1b")
            nc.vector.tensor_add(s1[:], pp[:], ts_c)
            q = tmp("qb")
            nc.vector.scalar_tensor_tensor(q[:], pt[:], -2.0, s1[:], AL.mult, AL.add)
            at = tmp("atb")
            nc.vector.tensor_scalar(at[:], ts_c, -0.5, 0.75, AL.mult, AL.add)
            ace = tmp("aceb")
            nc.vector.tensor_mul(ace[:], at[:], ce[:])
            q2 = tmp("q2b")
            nc.vector.tensor_mul(q2[:], q[:], q[:])
            junkb = tmp("junkbb")
            nc.vector.tensor_mul(junkb[:], ace[:], q2[:])
            nc.vector.tensor_reduce(
                FOC[si][:, img : img + 1],
                junkb[:],
                axis=mybir.AxisListType.X,
                op=AL.add,
            )
            mask = tmp("maskb")
            nc.vector.tensor_scalar(mask[:], ts_c, tau, None, AL.is_ge)
            nc.vector.tensor_reduce(
                CNT[si][:, img : img + 1],
                mask[:],
                axis=mybir.AxisListType.X,
                op=AL.add,
            )
            # masked -log(IoU(pred, tb))
            px2 = tmp("px2b")
            nc.vector.tensor_add(px2[:], px, pw)
            py2 = tmp("py2b")
            nc.vector.tensor_add(py2[:], py, ph)
            ix = tmp("ixb")
            nc.vector.tensor_tensor(ix[:], px2[:], tx2[:], AL.min)
            jx = tmp("jxb")
            nc.vector.tensor_max(jx[:], px, tbx)
            wI = tmp("wIb")
            nc.vector.tensor_sub(wI[:], ix[:], jx[:])
            iy = tmp("iyb")
            nc.vector.tensor_tensor(iy[:], py2[:], ty2[:], AL.min)
            jy = tmp("jyb")
            nc.vector.tensor_max(jy[:], py, tby)
            hI = tmp("hIb")
            nc.vector.tensor_sub(hI[:], iy[:], jy[:])
            hrI = tmp("hrIb")
            nc.vector.tensor_scalar(hrI[:], hI[:], 0.0, None, AL.max)
            interI = tmp("interIb")
            nc.vector.scalar_tensor_tensor(
                interI[:], wI[:], 0.0, hrI[:], AL.max, AL.mult
            )
            pa = tmp("pab")
            nc.vector.tensor_mul(pa[:], pw, ph)
            u1 = tmp("u1b")
            nc.vector.tensor_add(u1[:], pa[:], ta[:])
            u2 = tmp("u2b")
            nc.vector.tensor_sub(u2[:], u1[:], interI[:])
            lnIb = tmp("lnIbb")
            nc.scalar.activation(lnIb[:], interI[:], AF.Ln, bias=biasEps[:])
            lnUb = tmp("lnUbb")
            nc.scalar.activation(lnUb[:], u2[:], AF.Ln, bias=bias0[:])
            db = tmp("dbb")
            nc.vector.tensor_sub(db[:], lnUb[:], lnIb[:])
            junkc = tmp("junkcb")
            nc.vector.tensor_mul(junkc[:], db[:], mask[:])
            nc.vector.tensor_reduce(
                IOL[si][:, img : img + 1],
                junkc[:],
                axis=mybir.AxisListType.X,
                op=AL.add,
            )

    # ---- final per-core reduction -> (P, 8) ----
    nc.vector.memset(ACC[:], 0.0)
    for si in range(2):
        nc.vector.reduce_sum(
            ACC[:, 0 + si : 1 + si], FOC[si][:], axis=mybir.AxisListType.X
        )
        nc.vector.reduce_sum(
            ACC[:, 2 + si : 3 + si], CNT[si][:], axis=mybir.AxisListType.X
        )
        nc.vector.reduce_sum(
            ACC[:, 4 + si : 5 + si], IOL[si][:], axis=mybir.AxisListType.X
        )
    nc.gpsimd.dma_start(out_d, ACC[:])


def _get_nc():
    if "nc" not in _CACHE:
        _CACHE["nc"] = _build_kernel()
    return _CACHE["nc"]


def make_in_maps(fs_proposal, ss_proposal, anchors, ground_truth):
    anc = np.zeros((CPAD, 4), np.float32)
    anc[:C] = anchors
    fs = np.zeros((B, CPAD, 6), np.float32)
    fs[:, :C] = fs_proposal
    fs[:, C:, 4] = -60.0
    ss = np.zeros((B, CPAD, 6), np.float32)
    ss[:, :C] = ss_proposal
    ss[:, C:, 4] = -60.0
    gt = np.ascontiguousarray(ground_truth, np.float32)
    in_maps = []
    for c in range(8):
        sl = slice(c * PC, (c + 1) * PC)
        in_maps.append(
            {
                "anc": np.ascontiguousarray(anc[sl]),
                "gt": gt,
                "fs": np.ascontiguousarray(fs[:, sl]),
                "ss": np.ascontiguousarray(ss[:, sl]),
            }
        )
    return in_maps


def kernel(fs_proposal, ss_proposal, anchors, ground_truth):
    fs_proposal = np.ascontiguousarray(fs_proposal, np.float32)
    ss_proposal = np.ascontiguousarray(ss_proposal, np.float32)
    anchors = np.ascontiguousarray(anchors, np.float32)
    ground_truth = np.ascontiguousarray(ground_truth, np.float32)

    in_maps = make_in_maps(fs_proposal, ss_proposal, anchors, ground_truth)
    nc = _get_nc()
    res = run_bass_kernel_spmd(nc, in_maps, core_ids=list(range(8)))
    parts = np.stack([res.results[i]["out"] for i in range(8)])  # (8,128,8)
    tot = parts.sum(axis=(0, 1), dtype=np.float64)  # focF,focS,cntF,cntS,iolF,iolS
    fs_cnt = max(tot[2], 1.0)
    ss_cnt = max(tot[3], 1.0)
    loss = (
        tot[0] / (B * C) / fs_cnt
        + tot[1] / (B * C) / ss_cnt
        + tot[4] / fs_cnt
        + tot[5] / ss_cnt
    )
    return np.float32(loss)


# revision 33
# speedup vs baseline: 1.3635x; 1.3635x over previous
"""Trainium2 Bass kernel for AInnoFaceLoss (anchor-matching detection loss).

Sharding: anchors (C) are split across the 8 NeuronCores; every core
handles its 25088-anchor stripe against ALL 8 images' ground-truth
boxes.  This puts 8 images x 64 GT = 512 elements on the free axis of
every pairwise op (vs 64 for batch sharding), amortizing the ~58-cycle
DVE instruction overhead, while anchor coordinates stay per-partition
scalars (free operands of tensor_scalar / scalar_tensor_tensor).

Phase A per anchor-block t (128 anchors on partitions, 512 GT on free):
pairwise IoU, per-image max -> ts, one-hot(argmax) -> matched box tb.
The division inter/union runs as exp(ln - ln) on the Scalar engine's
LUT pipe (single combined ln+exp table; the allocator is patched so it
never reloads tables).  tb gathering runs on the otherwise-idle
TensorEngine: PE-transpose of the one-hot mask + tiny matmuls with the
GT coordinate lists.

Phase B (per image, anchors along free): sigmoid-focal terms via
ln/exp identities, threshold counts, masked -log(elementwise IoU).

Each core emits 6 partial sums; host combines (global counts, the
final four divisions) - order-free reductions, so the anchor
permutation from sharding never needs to be undone.
"""
from contextlib import ExitStack

import numpy as np

import concourse.bass as bass
import concourse.tile as tile
from concourse import bacc, mybir
from concourse.bass_utils import run_bass_kernel_spmd
from concourse.masks import make_identity

B, C, K = 8, 200000, 64
P = 128
NTC = 196  # anchor blocks per core
PC = P * NTC  # 25088 anchors per core
CPAD = 8 * PC  # 200704
G = B * K  # 512 gt across all images
FS_HI, SS_HI = 0.7, 0.5
DT = mybir.dt.float32
AL = mybir.AluOpType
AF = mybir.ActivationFunctionType

_CACHE = {}


def _patch_act_tables():
    """Keep ln/exp/abs only in the one table that holds all three;
    otherwise the allocator ping-pongs tables with a ~1.3us
    ACT_TABLE_LOAD between every Ln and Exp instruction."""
    from concourse import hw_specs

    orig = hw_specs.get_activation_tables

    def only_lnexp(arch):
        t = dict(orig(arch))
        key = "natural_log_exp_and_others"
        strip = {AF.Ln, AF.Exp, AF.Abs}
        for k in t:
            if k != key:
                t[k] = t[k] - strip
        return t

    bacc.get_activation_tables = only_lnexp


def _build_kernel():
    _patch_act_tables()
    nc = bacc.Bacc(
        "TRN2",
        target_bir_lowering=False,
        debug=False,
        enable_asserts=False,
        num_devices=8,
    )
    anc_d = nc.dram_tensor("anc", [PC, 4], DT, kind="ExternalInput").ap()
    gt_d = nc.dram_tensor("gt", [B, K, 4], DT, kind="ExternalInput").ap()
    fs_d = nc.dram_tensor("fs", [B, PC, 6], DT, kind="ExternalInput").ap()
    ss_d = nc.dram_tensor("ss", [B, PC, 6], DT, kind="ExternalInput").ap()
    out_d = nc.dram_tensor("out", [P, 8], DT, kind="ExternalOutput").ap()

    with tile.TileContext(nc) as tc:
        with ExitStack() as ctx:
            _body(ctx, tc, anc_d, gt_d, fs_d, ss_d, out_d)
    nc.compile()
    return nc


def _body(ctx, tc, anc_d, gt_d, fs_d, ss_d, out_d):
    nc = tc.nc

    persist = ctx.enter_context(tc.tile_pool(name="persist", bufs=1))
    psum = ctx.enter_context(tc.tile_pool(name="psum", bufs=1, space="PSUM"))
    psA = ctx.enter_context(tc.tile_pool(name="psA", bufs=2, space="PSUM"))
    atmp = ctx.enter_context(tc.tile_pool(name="atmp", bufs=3))
    btmp = ctx.enter_context(tc.tile_pool(name="btmp", bufs=1))
    bload = ctx.enter_context(tc.tile_pool(name="bload", bufs=1))

    bias0 = persist.tile([P, 1], DT, tag="bias0")
    nc.vector.memset(bias0[:], 0.0)
    biasEps = persist.tile([P, 1], DT, tag="biasEps")
    nc.vector.memset(biasEps[:], 1e-30)
    bias1 = persist.tile([P, 1], DT, tag="bias1")
    nc.vector.memset(bias1[:], 1.0)

    ident = persist.tile([P, P], DT, tag="ident")
    make_identity(nc, ident[:])

    # ---- anchors (anchor index within stripe = p*NTC + t) ----
    A = persist.tile([P, NTC, 4], DT, tag="A")
    nc.gpsimd.dma_start(A[:], anc_d.rearrange("(p t) c -> p t c", p=P))
    X2 = persist.tile([P, NTC], DT, tag="X2")
    Y2 = persist.tile([P, NTC], DT, tag="Y2")
    AR = persist.tile([P, NTC], DT, tag="AR")
    nc.vector.tensor_add(X2[:], A[:, :, 0], A[:, :, 2])
    nc.vector.tensor_add(Y2[:], A[:, :, 1], A[:, :, 3])
    nc.vector.tensor_mul(AR[:], A[:, :, 2], A[:, :, 3])

    # ---- gt broadcast across partitions: (128, 512) coordinate tiles ----
    grow = persist.tile([1, G * 4], DT, tag="grow")
    nc.gpsimd.dma_start(grow[:], gt_d.rearrange("b k c -> (b k c)")[None, :])
    ones = persist.tile([1, P], DT, tag="ones")
    nc.vector.memset(ones[:], 1.0)
    gps = psum.tile([P, G * 4], DT, tag="gps")
    for i in range(4):
        nc.tensor.matmul(
            gps[:, i * 512 : (i + 1) * 512], ones[:], grow[:, i * 512 : (i + 1) * 512]
        )
    gb = gps[:].rearrange("p (g c) -> p g c", g=G)
    GX = persist.tile([P, G], DT, tag="GX")
    GY = persist.tile([P, G], DT, tag="GY")
    GW = persist.tile([P, G], DT, tag="GW")
    GH = persist.tile([P, G], DT, tag="GH")
    nc.vector.tensor_copy(GX[:], gb[:, :, 0])
    nc.vector.tensor_copy(GY[:], gb[:, :, 1])
    nc.vector.tensor_copy(GW[:], gb[:, :, 2])
    nc.vector.tensor_copy(GH[:], gb[:, :, 3])
    BX2 = persist.tile([P, G], DT, tag="BX2")
    BY2 = persist.tile([P, G], DT, tag="BY2")
    SB = persist.tile([P, G], DT, tag="SB")
    nc.vector.tensor_add(BX2[:], GX[:], GW[:])
    nc.vector.tensor_add(BY2[:], GY[:], GH[:])
    nc.vector.tensor_mul(SB[:], GW[:], GH[:])

    # gt per-image coordinate lists for the tb matmuls, duplicated on
    # partitions 64-127 so either half can serve as matmul rhs: (128, B, 4)
    gtm = persist.tile([2 * K, B, 4], DT, tag="gtm")
    nc.gpsimd.dma_start(gtm[:K], gt_d.rearrange("b k c -> k b c"))
    nc.gpsimd.dma_start(gtm[K:], gt_d.rearrange("b k c -> k b c"))

    # ---- phase A outputs ----
    TS = persist.tile([P, NTC, B], DT, tag="TS")
    TB4 = persist.tile([P, NTC, B * 4], DT, tag="TB4")
    if not (DO_TB and DO_MM):
        nc.vector.memset(TB4[:], 1.0)

    # ---- phase A ----
    for t in range(NTC):
        ax1 = A[:, t, 0:1]
        ay1 = A[:, t, 1:2]
        ax2 = X2[:, t : t + 1]
        ay2 = Y2[:, t : t + 1]
        sa = AR[:, t : t + 1]

        t2 = atmp.tile([P, G], DT, tag="t2")
        nc.vector.tensor_scalar(t2[:], GX[:], ax1, None, AL.max)
        w = atmp.tile([P, G], DT, tag="w")
        nc.vector.scalar_tensor_tensor(w[:], BX2[:], ax2, t2[:], AL.min, AL.subtract)
        t4 = atmp.tile([P, G], DT, tag="t4")
        nc.vector.tensor_scalar(t4[:], GY[:], ay1, None, AL.max)
        h = atmp.tile([P, G], DT, tag="h")
        nc.vector.scalar_tensor_tensor(h[:], BY2[:], ay2, t4[:], AL.min, AL.subtract)
        hr = atmp.tile([P, G], DT, tag="hr")
        nc.vector.tensor_scalar(hr[:], h[:], 0.0, None, AL.max)
        inter = atmp.tile([P, G], DT, tag="inter")
        nc.vector.scalar_tensor_tensor(inter[:], w[:], 0.0, hr[:], AL.max, AL.mult)
        un = atmp.tile([P, G], DT, tag="un")
        nc.vector.scalar_tensor_tensor(un[:], SB[:], sa, inter[:], AL.add, AL.subtract)

        lnI = atmp.tile([P, G], DT, tag="lnI")
        nc.scalar.activation(lnI[:], inter[:], AF.Ln, bias=biasEps[:])
        lnU = atmp.tile([P, G], DT, tag="lnU")
        nc.scalar.activation(lnU[:], un[:], AF.Ln, bias=bias0[:])
        d = atmp.tile([P, G], DT, tag="d")
        nc.gpsimd.tensor_sub(d[:], lnI[:], lnU[:])
        iou = atmp.tile([P, G], DT, tag="iou")
        nc.scalar.activation(iou[:], d[:], AF.Exp, bias=bias0[:])

        iou3 = iou[:].rearrange("p (b k) -> p b k", b=B)
        nc.vector.tensor_reduce(TS[:, t, :], iou3, axis=mybir.AxisListType.X, op=AL.max)
        m = atmp.tile([P, G], DT, tag="m")
        tsb = TS[:, t : t + 1, :].rearrange("p o b -> p b o").to_broadcast([P, B, K])
        nc.vector.tensor_tensor(
            m[:].rearrange("p (b k) -> p b k", b=B), iou3, tsb, AL.is_ge
        )

        # tb = onehot @ gt, on the TensorEngine
        mts = atmp.tile([P, G], DT, tag="mts")
        for g in range(4):
            mtp = psA.tile([P, P], DT, tag="mtp", name="mtp")
            nc.tensor.transpose(mtp[:], m[:, g * P : (g + 1) * P], ident[:])
            nc.scalar.copy(mts[:, g * P : (g + 1) * P], mtp[:])
        if DO_MM:
            for img in range(B):
                g, half = img // 2, img % 2
                tbp = psA.tile([P, 4], DT, tag="tbp", name="tbp")
                nc.tensor.matmul(
                    tbp[:],
                    mts[half * K : (half + 1) * K, g * P : (g + 1) * P],
                    gtm[half * K : (half + 1) * K, img, :],
                )
                nc.scalar.copy(TB4[:, t, img * 4 : (img + 1) * 4], tbp[:])
        else:
            nc.vector.tensor_copy(TB4[:, t, 0:16], mts[:, 0:16])

    # ---- phase B ----
    ACC = persist.tile([P, 8], DT, tag="ACC")
    FOC = [persist.tile([P, B], DT, tag=f"FOC{i}", name=f"FOC{i}") for i in range(2)]
    CNT = [persist.tile([P, B], DT, tag=f"CNT{i}", name=f"CNT{i}") for i in range(2)]
    IOL = [persist.tile([P, B], DT, tag=f"IOL{i}", name=f"IOL{i}") for i in range(2)]

    F = NTC
    for img in range(B):
        ts_c = TS[:, :, img]
        tbx = TB4[:, :, img * 4 + 0]
        tby = TB4[:, :, img * 4 + 1]
        tbw = TB4[:, :, img * 4 + 2]
        tbh = TB4[:, :, img * 4 + 3]

        def tmp(tag):
            return btmp.tile([P, F], DT, tag=tag, name=tag)

        tx2 = tmp("tx2")
        nc.vector.tensor_add(tx2[:], tbx, tbw)
        ty2 = tmp("ty2")
        nc.vector.tensor_add(ty2[:], tby, tbh)
        ta = tmp("ta")
        nc.vector.tensor_mul(ta[:], tbw, tbh)

        for si, (src_d, tau) in enumerate(((fs_d, FS_HI), (ss_d, SS_HI))):
            pr = bload.tile([P, F, 6], DT, tag=f"prop{si}", name=f"prop{si}")
            nc.gpsimd.dma_start(pr[:], src_d[img].rearrange("(p t) c -> p t c", p=P))
            px = pr[:, :, 0]
            py = pr[:, :, 1]
            pw = pr[:, :, 2]
            ph = pr[:, :, 3]
            lg = pr[:, :, 4]

            # focal loss via ln/exp only:
            #   lp = ln(1+exp(-|l|)); softplus(l) = relu(l)+lp
            #   p = sigmoid(l) = exp(min(l,0) - lp)
            al = tmp("al")
            nc.scalar.activation(al[:], lg, AF.Abs, bias=bias0[:])
            ex = tmp("ex")
            nc.scalar.activation(ex[:], al[:], AF.Exp, bias=bias0[:], scale=-1.0)
            lp = tmp("lpb")
            nc.scalar.activation(lp[:], ex[:], AF.Ln, bias=bias1[:])
            parg = tmp("pargb")
            nc.vector.scalar_tensor_tensor(parg[:], lg, 0.0, lp[:], AL.min, AL.subtract)
            pp = tmp("ppb")
            nc.scalar.activation(pp[:], parg[:], AF.Exp, bias=bias0[:])
            sp = tmp("spb")
            nc.vector.scalar_tensor_tensor(sp[:], lg, 0.0, lp[:], AL.max, AL.add)
            lt = tmp("ltb")
            nc.vector.tensor_mul(lt[:], lg, ts_c)
            ce = tmp("ceb")
            nc.vector.tensor_sub(ce[:], sp[:], lt[:])
            pt = tmp("ptb")
            nc.vector.tensor_mul(pt[:], pp[:], ts_c)
            s1 = tmp("s1b")
            nc.vector.tensor_add(s1[:], pp[:], ts_c)
            q = tmp("qb")
            nc.vector.scalar_tensor_tensor(q[:], pt[:], -2.0, s1[:], AL.mult, AL.add)
            at = tmp("atb")
            nc.vector.tensor_scalar(at[:], ts_c, -0.5, 0.75, AL.mult, AL.add)
            ace = tmp("aceb")
            nc.vector.tensor_mul(ace[:], at[:], ce[:])
            q2 = tmp("q2b")
            nc.vector.tensor_mul(q2[:], q[:], q[:])
            junkb = tmp("junkbb")
            nc.vector.tensor_mul(junkb[:], ace[:], q2[:])
            nc.vector.tensor_reduce(
                FOC[si][:, img : img + 1],
                junkb[:],
                axis=mybir.AxisListType.X,
                op=AL.add,
            )
            mask = tmp("maskb")
            nc.vector.tensor_scalar(mask[:], ts_c, tau, None, AL.is_ge)
            nc.vector.tensor_reduce(
                CNT[si][:, img : img + 1],
                mask[:],
                axis=mybir.AxisListType.X,
                op=AL.add,
            )
            # masked -log(IoU(pred, tb))
            px2 = tmp("px2b")
            nc.vector.tensor_add(px2[:], px, pw)
            py2 = tmp("py2b")
            nc.vector.tensor_add(py2[:], py, ph)
            ix = tmp("ixb")
            nc.vector.tensor_tensor(ix[:], px2[:], tx2[:], AL.min)
            jx = tmp("jxb")
            nc.vector.tensor_max(jx[:], px, tbx)
            wI = tmp("wIb")
            nc.vector.tensor_sub(wI[:], ix[:], jx[:])
            iy = tmp("iyb")
            nc.vector.tensor_tensor(iy[:], py2[:], ty2[:], AL.min)
            jy = tmp("jyb")
            nc.vector.tensor_max(jy[:], py, tby)
            hI = tmp("hIb")
            nc.vector.tensor_sub(hI[:], iy[:], jy[:])
            hrI = tmp("hrIb")
            nc.vector.tensor_scalar(hrI[:], hI[:], 0.0, None, AL.max)
            interI = tmp("interIb")
            nc.vector.scalar_tensor_tensor(
                interI[:], wI[:], 0.0, hrI[:], AL.max, AL.mult
            )
            pa = tmp("pab")
            nc.vector.tensor_mul(pa[:], pw, ph)
            u1 = tmp("u1b")
            nc.vector.tensor_add(u1[:], pa[:], ta[:])
            u2 = tmp("u2b")
            nc.vector.tensor_sub(u2[:], u1[:], interI[:])
            lnIb = tmp("lnIbb")
            nc.scalar.activation(lnIb[:], interI[:], AF.Ln, bias=biasEps[:])
            lnUb = tmp("lnUbb")
            nc.scalar.activation(lnUb[:], u2[:], AF.Ln, bias=bias0[:])
            db = tmp("dbb")
            nc.vector.tensor_sub(db[:], lnUb[:], lnIb[:])
            junkc = tmp("junkcb")
            nc.vector.tensor_mul(junkc[:], db[:], mask[:])
            nc.vector.tensor_reduce(
                IOL[si][:, img : img + 1],
                junkc[:],
                axis=mybir.AxisListType.X,
                op=AL.add,
            )

    # ---- final per-core reduction -> (P, 8) ----
    nc.vector.memset(ACC[:], 0.0)
    for si in range(2):
        nc.vector.reduce_sum(
            ACC[:, 0 + si : 1 + si], FOC[si][:], axis=mybir.AxisListType.X
        )
        nc.vector.reduce_sum(
            ACC[:, 2 + si : 3 + si], CNT[si][:], axis=mybir.AxisListType.X
        )
        nc.vector.reduce_sum(
            ACC[:, 4 + si : 5 + si], IOL[si][:], axis=mybir.AxisListType.X
        )
    nc.gpsimd.dma_start(out_d, ACC[:])


def _get_nc():
    if "nc" not in _CACHE:
        _CACHE["nc"] = _build_kernel()
    return _CACHE["nc"]


def make_in_maps(fs_proposal, ss_proposal, anchors, ground_truth):
    anc = np.zeros((CPAD, 4), np.float32)
    anc[:C] = anchors
    fs = np.zeros((B, CPAD, 6), np.float32)
    fs[:, :C] = fs_proposal
    fs[:, C:, 4] = -60.0
    ss = np.zeros((B, CPAD, 6), np.float32)
    ss[:, :C] = ss_proposal
    ss[:, C:, 4] = -60.0
    gt = np.ascontiguousarray(ground_truth, np.float32)
    in_maps = []
    for c in range(8):
        sl = slice(c * PC, (c + 1) * PC)
        in_maps.append(
            {
                "anc": np.ascontiguousarray(anc[sl]),
                "gt": gt,
                "fs": np.ascontiguousarray(fs[:, sl]),
                "ss": np.ascontiguousarray(ss[:, sl]),
            }
        )
    return in_maps


def kernel(fs_proposal, ss_proposal, anchors, ground_truth):
    fs_proposal = np.ascontiguousarray(fs_proposal, np.float32)
    ss_proposal = np.ascontiguousarray(ss_proposal, np.float32)
    anchors = np.ascontiguousarray(anchors, np.float32)
    ground_truth = np.ascontiguousarray(ground_truth, np.float32)

    in_maps = make_in_maps(fs_proposal, ss_proposal, anchors, ground_truth)
    nc = _get_nc()
    res = run_bass_kernel_spmd(nc, in_maps, core_ids=list(range(8)))
    parts = np.stack([res.results[i]["out"] for i in range(8)])  # (8,128,8)
    tot = parts.sum(axis=(0, 1), dtype=np.float64)  # focF,focS,cntF,cntS,iolF,iolS
    fs_cnt = max(tot[2], 1.0)
    ss_cnt = max(tot[3], 1.0)
    loss = (
        tot[0] / (B * C) / fs_cnt
        + tot[1] / (B * C) / ss_cnt
        + tot[4] / fs_cnt
        + tot[5] / ss_cnt
    )
    return np.float32(loss)


# revision 34
# speedup vs baseline: 1.6370x; 1.2006x over previous
"""Trainium2 Bass kernel for AInnoFaceLoss (anchor-matching detection loss).

Sharding: anchors (C) are split across the 8 NeuronCores; every core
handles its 25088-anchor stripe against ALL 8 images' ground-truth
boxes.  This puts 8 images x 64 GT = 512 elements on the free axis of
every pairwise op (vs 64 for batch sharding), amortizing the ~58-cycle
DVE instruction overhead, while anchor coordinates stay per-partition
scalars (free operands of tensor_scalar / scalar_tensor_tensor).

Phase A per anchor-block t (128 anchors on partitions, 512 GT on free):
pairwise IoU, per-image max -> ts, one-hot(argmax) -> matched box tb.
The division inter/union runs as exp(ln - ln) on the Scalar engine's
LUT pipe (single combined ln+exp table; the allocator is patched so it
never reloads tables).  tb gathering runs on the otherwise-idle
TensorEngine: PE-transpose of the one-hot mask + tiny matmuls with the
GT coordinate lists.

Phase B (per image, anchors along free): sigmoid-focal terms via
ln/exp identities, threshold counts, masked -log(elementwise IoU).

Each core emits 6 partial sums; host combines (global counts, the
final four divisions) - order-free reductions, so the anchor
permutation from sharding never needs to be undone.
"""
from contextlib import ExitStack

import numpy as np

import concourse.bass as bass
import concourse.tile as tile
from concourse import bacc, mybir
from concourse.bass_utils import run_bass_kernel_spmd
from concourse.masks import make_identity

B, C, K = 8, 200000, 64
P = 128
NTC = 196  # anchor blocks per core
PC = P * NTC  # 25088 anchors per core
CPAD = 8 * PC  # 200704
G = B * K  # 512 gt across all images
FS_HI, SS_HI = 0.7, 0.5
DT = mybir.dt.float32
AL = mybir.AluOpType
AF = mybir.ActivationFunctionType

_CACHE = {}


def _patch_act_tables():
    """Keep ln/exp/abs only in the one table that holds all three;
    otherwise the allocator ping-pongs tables with a ~1.3us
    ACT_TABLE_LOAD between every Ln and Exp instruction."""
    from concourse import hw_specs

    orig = hw_specs.get_activation_tables

    def only_lnexp(arch):
        t = dict(orig(arch))
        key = "natural_log_exp_and_others"
        strip = {AF.Ln, AF.Exp, AF.Abs}
        for k in t:
            if k != key:
                t[k] = t[k] - strip
        return t

    bacc.get_activation_tables = only_lnexp


def _build_kernel():
    _patch_act_tables()
    nc = bacc.Bacc(
        "TRN2",
        target_bir_lowering=False,
        debug=False,
        enable_asserts=False,
        num_devices=8,
    )
    anc_d = nc.dram_tensor("anc", [PC, 4], DT, kind="ExternalInput").ap()
    gt_d = nc.dram_tensor("gt", [B, K, 4], DT, kind="ExternalInput").ap()
    fs_d = nc.dram_tensor("fs", [B, PC, 6], DT, kind="ExternalInput").ap()
    ss_d = nc.dram_tensor("ss", [B, PC, 6], DT, kind="ExternalInput").ap()
    out_d = nc.dram_tensor("out", [P, 8], DT, kind="ExternalOutput").ap()

    with tile.TileContext(nc) as tc:
        with ExitStack() as ctx:
            _body(ctx, tc, anc_d, gt_d, fs_d, ss_d, out_d)
    nc.compile()
    return nc


def _body(ctx, tc, anc_d, gt_d, fs_d, ss_d, out_d):
    nc = tc.nc

    persist = ctx.enter_context(tc.tile_pool(name="persist", bufs=1))
    psum = ctx.enter_context(tc.tile_pool(name="psum", bufs=1, space="PSUM"))
    psA = ctx.enter_context(tc.tile_pool(name="psA", bufs=2, space="PSUM"))
    atmp = ctx.enter_context(tc.tile_pool(name="atmp", bufs=3))
    btmp = ctx.enter_context(tc.tile_pool(name="btmp", bufs=1))
    bload = ctx.enter_context(tc.tile_pool(name="bload", bufs=1))

    bias0 = persist.tile([P, 1], DT, tag="bias0")
    nc.vector.memset(bias0[:], 0.0)
    biasEps = persist.tile([P, 1], DT, tag="biasEps")
    nc.vector.memset(biasEps[:], 1e-30)
    bias1 = persist.tile([P, 1], DT, tag="bias1")
    nc.vector.memset(bias1[:], 1.0)

    ident = persist.tile([P, P], DT, tag="ident")
    make_identity(nc, ident[:])

    # ---- anchors (anchor index within stripe = p*NTC + t) ----
    A = persist.tile([P, NTC, 4], DT, tag="A")
    nc.gpsimd.dma_start(A[:], anc_d.rearrange("(p t) c -> p t c", p=P))
    X2 = persist.tile([P, NTC], DT, tag="X2")
    Y2 = persist.tile([P, NTC], DT, tag="Y2")
    AR = persist.tile([P, NTC], DT, tag="AR")
    nc.vector.tensor_add(X2[:], A[:, :, 0], A[:, :, 2])
    nc.vector.tensor_add(Y2[:], A[:, :, 1], A[:, :, 3])
    nc.vector.tensor_mul(AR[:], A[:, :, 2], A[:, :, 3])

    # ---- gt broadcast across partitions: (128, 512) coordinate tiles ----
    grow = persist.tile([1, G * 4], DT, tag="grow")
    nc.gpsimd.dma_start(grow[:], gt_d.rearrange("b k c -> (b k c)")[None, :])
    ones = persist.tile([1, P], DT, tag="ones")
    nc.vector.memset(ones[:], 1.0)
    gps = psum.tile([P, G * 4], DT, tag="gps")
    for i in range(4):
        nc.tensor.matmul(
            gps[:, i * 512 : (i + 1) * 512], ones[:], grow[:, i * 512 : (i + 1) * 512]
        )
    gb = gps[:].rearrange("p (g c) -> p g c", g=G)
    GX = persist.tile([P, G], DT, tag="GX")
    GY = persist.tile([P, G], DT, tag="GY")
    GW = persist.tile([P, G], DT, tag="GW")
    GH = persist.tile([P, G], DT, tag="GH")
    nc.vector.tensor_copy(GX[:], gb[:, :, 0])
    nc.vector.tensor_copy(GY[:], gb[:, :, 1])
    nc.vector.tensor_copy(GW[:], gb[:, :, 2])
    nc.vector.tensor_copy(GH[:], gb[:, :, 3])
    BX2 = persist.tile([P, G], DT, tag="BX2")
    BY2 = persist.tile([P, G], DT, tag="BY2")
    SB = persist.tile([P, G], DT, tag="SB")
    nc.vector.tensor_add(BX2[:], GX[:], GW[:])
    nc.vector.tensor_add(BY2[:], GY[:], GH[:])
    nc.vector.tensor_mul(SB[:], GW[:], GH[:])

    # gt per-image coordinate lists for the tb matmuls, duplicated on
    # partitions 64-127 so either half can serve as matmul rhs: (128, B, 4)
    gtm = persist.tile([2 * K, B, 4], DT, tag="gtm")
    nc.gpsimd.dma_start(gtm[:K], gt_d.rearrange("b k c -> k b c"))
    nc.gpsimd.dma_start(gtm[K:], gt_d.rearrange("b k c -> k b c"))

    # ---- phase A outputs ----
    TS = persist.tile([P, NTC, B], DT, tag="TS")
    TB4 = persist.tile([P, NTC, B * 4], DT, tag="TB4")
    if not (DO_TB and DO_MM):
        nc.vector.memset(TB4[:], 1.0)

    # ---- phase A ----
    for t in range(NTC):
        ax1 = A[:, t, 0:1]
        ay1 = A[:, t, 1:2]
        ax2 = X2[:, t : t + 1]
        ay2 = Y2[:, t : t + 1]
        sa = AR[:, t : t + 1]

        t2 = atmp.tile([P, G], DT, tag="t2")
        nc.vector.tensor_scalar(t2[:], GX[:], ax1, None, AL.max)
        w = atmp.tile([P, G], DT, tag="w")
        nc.vector.scalar_tensor_tensor(w[:], BX2[:], ax2, t2[:], AL.min, AL.subtract)
        t4 = atmp.tile([P, G], DT, tag="t4")
        nc.vector.tensor_scalar(t4[:], GY[:], ay1, None, AL.max)
        h = atmp.tile([P, G], DT, tag="h")
        nc.vector.scalar_tensor_tensor(h[:], BY2[:], ay2, t4[:], AL.min, AL.subtract)
        hr = atmp.tile([P, G], DT, tag="hr")
        nc.vector.tensor_scalar(hr[:], h[:], 0.0, None, AL.max)
        inter = atmp.tile([P, G], DT, tag="inter")
        nc.vector.scalar_tensor_tensor(inter[:], w[:], 0.0, hr[:], AL.max, AL.mult)
        un = atmp.tile([P, G], DT, tag="un")
        nc.vector.scalar_tensor_tensor(un[:], SB[:], sa, inter[:], AL.add, AL.subtract)

        lnI = atmp.tile([P, G], DT, tag="lnI")
        nc.scalar.activation(lnI[:], inter[:], AF.Ln, bias=biasEps[:])
        lnU = atmp.tile([P, G], DT, tag="lnU")
        nc.scalar.activation(lnU[:], un[:], AF.Ln, bias=bias0[:])
        d = atmp.tile([P, G], DT, tag="d")
        nc.gpsimd.tensor_sub(d[:], lnI[:], lnU[:])
        iou = atmp.tile([P, G], DT, tag="iou")
        nc.scalar.activation(iou[:], d[:], AF.Exp, bias=bias0[:])

        iou3 = iou[:].rearrange("p (b k) -> p b k", b=B)
        nc.vector.tensor_reduce(TS[:, t, :], iou3, axis=mybir.AxisListType.X, op=AL.max)
        m = atmp.tile([P, G], DT, tag="m")
        tsb = TS[:, t : t + 1, :].rearrange("p o b -> p b o").to_broadcast([P, B, K])
        nc.vector.tensor_tensor(
            m[:].rearrange("p (b k) -> p b k", b=B), iou3, tsb, AL.is_ge
        )

        # tb = onehot @ gt, on the TensorEngine
        mts = atmp.tile([P, G], DT, tag="mts")
        for g in range(4):
            mtp = psA.tile([P, P], DT, tag="mtp", name="mtp")
            nc.tensor.transpose(mtp[:], m[:, g * P : (g + 1) * P], ident[:])
            nc.scalar.copy(mts[:, g * P : (g + 1) * P], mtp[:])
        if DO_MM:
            for img in range(B):
                g, half = img // 2, img % 2
                tbp = psA.tile([P, 4], DT, tag="tbp", name="tbp")
                nc.tensor.matmul(
                    tbp[:],
                    mts[half * K : (half + 1) * K, g * P : (g + 1) * P],
                    gtm[half * K : (half + 1) * K, img, :],
                )
                nc.scalar.copy(TB4[:, t, img * 4 : (img + 1) * 4], tbp[:])
        else:
            nc.vector.tensor_copy(TB4[:, t, 0:16], mts[:, 0:16])

    # ---- phase B ----
    ACC = persist.tile([P, 8], DT, tag="ACC")
    FOC = [persist.tile([P, B], DT, tag=f"FOC{i}", name=f"FOC{i}") for i in range(2)]
    CNT = [persist.tile([P, B], DT, tag=f"CNT{i}", name=f"CNT{i}") for i in range(2)]
    IOL = [persist.tile([P, B], DT, tag=f"IOL{i}", name=f"IOL{i}") for i in range(2)]

    F = NTC
    for img in range(B):
        ts_c = TS[:, :, img]
        tbx = TB4[:, :, img * 4 + 0]
        tby = TB4[:, :, img * 4 + 1]
        tbw = TB4[:, :, img * 4 + 2]
        tbh = TB4[:, :, img * 4 + 3]

        def tmp(tag):
            return btmp.tile([P, F], DT, tag=tag, name=tag)

        tx2 = tmp("tx2")
        nc.vector.tensor_add(tx2[:], tbx, tbw)
        ty2 = tmp("ty2")
        nc.vector.tensor_add(ty2[:], tby, tbh)
        ta = tmp("ta")
        nc.vector.tensor_mul(ta[:], tbw, tbh)

        for si, (src_d, tau) in enumerate(((fs_d, FS_HI), (ss_d, SS_HI))):
            pr = bload.tile([P, F, 6], DT, tag=f"prop{si}", name=f"prop{si}")
            nc.gpsimd.dma_start(pr[:], src_d[img].rearrange("(p t) c -> p t c", p=P))
            px = pr[:, :, 0]
            py = pr[:, :, 1]
            pw = pr[:, :, 2]
            ph = pr[:, :, 3]
            lg = pr[:, :, 4]

            # focal loss via ln/exp only:
            #   lp = ln(1+exp(-|l|)); softplus(l) = relu(l)+lp
            #   p = sigmoid(l) = exp(min(l,0) - lp)
            al = tmp(f"al{si}")
            nc.scalar.activation(al[:], lg, AF.Abs, bias=bias0[:])
            ex = tmp(f"ex{si}")
            nc.scalar.activation(ex[:], al[:], AF.Exp, bias=bias0[:], scale=-1.0)
            lp = tmp(f"lp{si}")
            nc.scalar.activation(lp[:], ex[:], AF.Ln, bias=bias1[:])
            parg = tmp(f"parg{si}")
            nc.vector.scalar_tensor_tensor(parg[:], lg, 0.0, lp[:], AL.min, AL.subtract)
            pp = tmp(f"pp{si}")
            nc.scalar.activation(pp[:], parg[:], AF.Exp, bias=bias0[:])
            sp = tmp(f"sp{si}")
            nc.vector.scalar_tensor_tensor(sp[:], lg, 0.0, lp[:], AL.max, AL.add)
            lt = tmp(f"lt{si}")
            nc.vector.tensor_mul(lt[:], lg, ts_c)
            ce = tmp(f"ce{si}")
            nc.vector.tensor_sub(ce[:], sp[:], lt[:])
            pt = tmp(f"pt{si}")
            nc.vector.tensor_mul(pt[:], pp[:], ts_c)
            s1 = tmp(f"s1{si}")
            nc.vector.tensor_add(s1[:], pp[:], ts_c)
            q = tmp(f"q{si}")
            nc.vector.scalar_tensor_tensor(q[:], pt[:], -2.0, s1[:], AL.mult, AL.add)
            at = tmp(f"at{si}")
            nc.vector.tensor_scalar(at[:], ts_c, -0.5, 0.75, AL.mult, AL.add)
            ace = tmp(f"ace{si}")
            nc.vector.tensor_mul(ace[:], at[:], ce[:])
            q2 = tmp(f"q2{si}")
            nc.vector.tensor_mul(q2[:], q[:], q[:])
            junkb = tmp(f"junkb{si}")
            nc.vector.tensor_mul(junkb[:], ace[:], q2[:])
            nc.vector.tensor_reduce(
                FOC[si][:, img : img + 1],
                junkb[:],
                axis=mybir.AxisListType.X,
                op=AL.add,
            )
            mask = tmp(f"mask{si}")
            nc.vector.tensor_scalar(mask[:], ts_c, tau, None, AL.is_ge)
            nc.vector.tensor_reduce(
                CNT[si][:, img : img + 1],
                mask[:],
                axis=mybir.AxisListType.X,
                op=AL.add,
            )
            # masked -log(IoU(pred, tb))
            px2 = tmp(f"px2{si}")
            nc.vector.tensor_add(px2[:], px, pw)
            py2 = tmp(f"py2{si}")
            nc.vector.tensor_add(py2[:], py, ph)
            ix = tmp(f"ix{si}")
            nc.vector.tensor_tensor(ix[:], px2[:], tx2[:], AL.min)
            jx = tmp(f"jx{si}")
            nc.vector.tensor_max(jx[:], px, tbx)
            wI = tmp(f"wI{si}")
            nc.vector.tensor_sub(wI[:], ix[:], jx[:])
            iy = tmp(f"iy{si}")
            nc.vector.tensor_tensor(iy[:], py2[:], ty2[:], AL.min)
            jy = tmp(f"jy{si}")
            nc.vector.tensor_max(jy[:], py, tby)
            hI = tmp(f"hI{si}")
            nc.vector.tensor_sub(hI[:], iy[:], jy[:])
            hrI = tmp(f"hrI{si}")
            nc.vector.tensor_scalar(hrI[:], hI[:], 0.0, None, AL.max)
            interI = tmp(f"interI{si}")
            nc.vector.scalar_tensor_tensor(
                interI[:], wI[:], 0.0, hrI[:], AL.max, AL.mult
            )
            pa = tmp(f"pa{si}")
            nc.vector.tensor_mul(pa[:], pw, ph)
            u1 = tmp(f"u1{si}")
            nc.vector.tensor_add(u1[:], pa[:], ta[:])
            u2 = tmp(f"u2{si}")
            nc.vector.tensor_sub(u2[:], u1[:], interI[:])
            lnIb = tmp(f"lnIb{si}")
            nc.scalar.activation(lnIb[:], interI[:], AF.Ln, bias=biasEps[:])
            lnUb = tmp(f"lnUb{si}")
            nc.scalar.activation(lnUb[:], u2[:], AF.Ln, bias=bias0[:])
            db = tmp(f"db{si}")
            nc.vector.tensor_sub(db[:], lnUb[:], lnIb[:])
            junkc = tmp(f"junkc{si}")
            nc.vector.tensor_mul(junkc[:], db[:], mask[:])
            nc.vector.tensor_reduce(
                IOL[si][:, img : img + 1],
                junkc[:],
                axis=mybir.AxisListType.X,
                op=AL.add,
            )

    # ---- final per-core reduction -> (P, 8) ----
    nc.vector.memset(ACC[:], 0.0)
    for si in range(2):
        nc.vector.reduce_sum(
            ACC[:, 0 + si : 1 + si], FOC[si][:], axis=mybir.AxisListType.X
        )
        nc.vector.reduce_sum(
            ACC[:, 2 + si : 3 + si], CNT[si][:], axis=mybir.AxisListType.X
        )
        nc.vector.reduce_sum(
            ACC[:, 4 + si : 5 + si], IOL[si][:], axis=mybir.AxisListType.X
        )
    nc.gpsimd.dma_start(out_d, ACC[:])


def _get_nc():
    if "nc" not in _CACHE:
        _CACHE["nc"] = _build_kernel()
    return _CACHE["nc"]


def make_in_maps(fs_proposal, ss_proposal, anchors, ground_truth):
    anc = np.zeros((CPAD, 4), np.float32)
    anc[:C] = anchors
    fs = np.zeros((B, CPAD, 6), np.float32)
    fs[:, :C] = fs_proposal
    fs[:, C:, 4] = -60.0
    ss = np.zeros((B, CPAD, 6), np.float32)
    ss[:, :C] = ss_proposal
    ss[:, C:, 4] = -60.0
    gt = np.ascontiguousarray(ground_truth, np.float32)
    in_maps = []
    for c in range(8):
        sl = slice(c * PC, (c + 1) * PC)
        in_maps.append(
            {
                "anc": np.ascontiguousarray(anc[sl]),
                "gt": gt,
                "fs": np.ascontiguousarray(fs[:, sl]),
                "ss": np.ascontiguousarray(ss[:, sl]),
            }
        )
    return in_maps


def kernel(fs_proposal, ss_proposal, anchors, ground_truth):
    fs_proposal = np.ascontiguousarray(fs_proposal, np.float32)
    ss_proposal = np.ascontiguousarray(ss_proposal, np.float32)
    anchors = np.ascontiguousarray(anchors, np.float32)
    ground_truth = np.ascontiguousarray(ground_truth, np.float32)

    in_maps = make_in_maps(fs_proposal, ss_proposal, anchors, ground_truth)
    nc = _get_nc()
    res = run_bass_kernel_spmd(nc, in_maps, core_ids=list(range(8)))
    parts = np.stack([res.results[i]["out"] for i in range(8)])  # (8,128,8)
    tot = parts.sum(axis=(0, 1), dtype=np.float64)  # focF,focS,cntF,cntS,iolF,iolS
    fs_cnt = max(tot[2], 1.0)
    ss_cnt = max(tot[3], 1.0)
    loss = (
        tot[0] / (B * C) / fs_cnt
        + tot[1] / (B * C) / ss_cnt
        + tot[4] / fs_cnt
        + tot[5] / ss_cnt
    )
    return np.float32(loss)
